# revision 1
# baseline (speedup 1.0000x reference)
"""Trainium2 Bass kernel: batched soft 3-SAT circuit evaluation.

out[b, c] = 1 - prod_k z[c,k],  z = (sign>0 ? 1-x : x)[idx],
x = sigmoid(emb[0]).  Every batch row is identical (input_idx is all
zeros, the embedding has a single row, and jnp.take clamps OOB), so the
device computes each clause result once and broadcast-writes the rows.

Sharding: clauses split across 8 NeuronCores (5250 each, padded 5376).
Host work is index-layout prep only (fold sign into a combined table
index, pad, order literals chunk-major, wrap into the 16-partition
GPSIMD gather layout) plus concatenation of per-core outputs.

Per-core device pipeline (H = 4 column chunks of 1344 cols):
  prologue (4 col-quarters, two HWDGE rings): broadcast-load emb row
    into raw[128, NV]; ACT sigmoid -> x table half; DVE (x*-1)+1 ->
    1-x table half.  Combined table tab[128, 2*NV].
  per chunk h:
    - GPSIMD ap_gather: z[128, 512] literals (8 Q7 groups x 168 clauses)
    - DVE: r = 1 - z0*z1*z2  [128, 168] (replicated within each
      16-partition group)
    - PE: per group g a [K=16]x[M=128]x[N=168] matmul with lhsT=1/16
      broadcasts group g's row into all 128 partitions of PSUM (bitwise
      exact: sum of 16 identical values * 1/16)
    - ACT: copy PSUM -> SBUF bcast tile [128, 8*168]
    - 8 row-block DMAs bcast -> out[128b:128b+128, 1344h:1344h+1344]
      (5.4KB descriptors), alternating the sync/scalar HWDGE rings.
"""

import numpy as np

NV = 10000
C_TOTAL = 42000
KLIT = 3
B = 1024
NCORES = 8
C_CORE = C_TOTAL // NCORES     # 5250
GROUPS = 8                     # Q7 cores / 16-partition groups
C_PAD = 5376                   # padded clauses per core
CPGS = [168, 168, 168, 84]     # clauses per (group, Q7-chunk)
H = len(CPGS)
C_CHUNKS = [8 * c for c in CPGS]          # output cols per Q7 chunk
C_OFFS = [sum(C_CHUNKS[:h]) for h in range(H)]
LPCS = [c * KLIT for c in CPGS]           # real literals per (g, chunk)
LPC_PADS = [-(-l // 32) * 32 for l in LPCS]   # pad to 32 (2-col align)
COLS_HS = [l // 16 for l in LPC_PADS]     # idx cols per chunk
COL_OFFS = [sum(COLS_HS[:h]) for h in range(H)]
IDX_COLS = sum(COLS_HS)
PBLK = 256                     # PSUM cols reserved per group block

# PE-gathered tail: the last 672 output cols are gathered on the tensor
# engine via one-hot radix matmuls while the Q7 cores work the rest.
PE_C = C_PAD - sum(C_CHUNKS)   # 672 clauses
PE_OFF = sum(C_CHUNKS)         # col offset 4704
PE_L = PE_C * KLIT             # 2016 literals
PE_LP = 2048                   # padded to 4 tiles of 512
PE_TILES = PE_LP // 512
RADIX = 128                    # idx' = 128*hi + lo; hi < 157, lo < 128

_CACHE = {}


def _build():
    import concourse.bass as bass
    import concourse.tile as tile
    from concourse import bacc, mybir
    from contextlib import ExitStack

    f32 = mybir.dt.float32
    AF = mybir.ActivationFunctionType
    OP = mybir.AluOpType

    nc = bacc.Bacc("TRN2", target_bir_lowering=False, debug=False,
                   num_devices=NCORES)
    emb_d = nc.dram_tensor("emb", [1, NV], f32, kind="ExternalInput")
    idx_d = nc.dram_tensor("idxw", [128, IDX_COLS], mybir.dt.int16,
                           kind="ExternalInput")
    hia_d = nc.dram_tensor("hia", [1, PE_LP], f32, kind="ExternalInput")
    hib_d = nc.dram_tensor("hib", [1, PE_LP], f32, kind="ExternalInput")
    lo_d = nc.dram_tensor("lo", [1, PE_LP], f32, kind="ExternalInput")
    out_d = nc.dram_tensor("out", [B, C_PAD], f32, kind="ExternalOutput")

    with tile.TileContext(nc) as tc, ExitStack() as ctx:
        const = ctx.enter_context(tc.tile_pool(name="const", bufs=1))
        work = ctx.enter_context(tc.tile_pool(name="work", bufs=2))
        psum = ctx.enter_context(
            tc.tile_pool(name="psum", bufs=1, space="PSUM"))
        pepsum = ctx.enter_context(
            tc.tile_pool(name="pepsum", bufs=2, space="PSUM"))
        dpool = ctx.enter_context(
            tc.tile_pool(name="dram", bufs=1, space="DRAM"))

        idx_sb = const.tile([128, IDX_COLS], mybir.dt.int16)

        # selector E[:, g, :]: E[k, g, m] = 1/16 iff k//16 == g; matmul
        # with it averages each group's 16 identical partition rows into
        # all 128 output partitions (bitwise exact).
        sel = const.tile([128, GROUPS, 128], f32)
        nc.vector.memset(sel[:], 1.0 / 16.0)
        # keep 1/16 only where 0 <= p - 16g <= 15, i.e. g == p//16
        nc.gpsimd.affine_select(sel[:, :, :], sel[:, :, :],
                                pattern=[[-16, GROUPS], [0, 128]],
                                compare_op=OP.is_ge, fill=0.0,
                                base=0, channel_multiplier=1)
        nc.gpsimd.affine_select(sel[:, :, :], sel[:, :, :],
                                pattern=[[16, GROUPS], [0, 128]],
                                compare_op=OP.is_ge, fill=0.0,
                                base=15, channel_multiplier=-1)

        # table padded to RADIX*157 = 20096 so the PE radix view is in
        # bounds; tail memset keeps the X2 copy finite
        tab = const.tile([128, 157 * RADIX], f32)
        nc.vector.memset(tab[:, 2 * NV:157 * RADIX], 0.0)
        rings = [nc.sync, nc.scalar]
        NQ = 8
        q = NV // NQ
        with tc.tile_pool(name="rawp", bufs=1) as rawp:
            raw = rawp.tile([128, NV], f32)
            # broadcast-load eighths alternate sync HWDGE / gpsimd
            # SWDGE: two queues give aggregate HBM-read rate, and the
            # scalar ring stays clear so ACT isn't delayed by dispatch
            for c in range(NQ):
                eng = nc.sync if c % 2 == 0 else nc.gpsimd
                eng.dma_start(
                    out=raw[:, c * q:(c + 1) * q],
                    in_=bass.AP(tensor=emb_d, offset=c * q,
                                ap=[[0, 128], [1, q]]))
            nc.gpsimd.dma_start(out=idx_sb[:], in_=idx_d[:, :])
            for c in range(NQ):
                sl = slice(c * q, (c + 1) * q)
                xs = slice(NV + c * q, NV + (c + 1) * q)
                nc.scalar.activation(tab[:, xs], raw[:, sl], AF.Sigmoid)
                # 1 - x on DVE, overlaps ACT of the next eighth
                nc.vector.tensor_scalar(tab[:, sl], tab[:, xs], -1.0,
                                        1.0, OP.mult, OP.add)

        # ---- PE-gather tail: one-hot inputs and table radix view ----
        hi_bc = const.tile([128, PE_LP], f32)
        hib_bc = const.tile([128, PE_LP], f32)
        lo_bc = const.tile([128, PE_LP], f32)
        for src_d, dst in ((hia_d, hi_bc), (hib_d, hib_bc), (lo_d, lo_bc)):
            nc.gpsimd.dma_start(
                out=dst[:],
                in_=bass.AP(tensor=src_d, offset=0,
                            ap=[[0, 128], [1, PE_LP]]))
        iota_i = const.tile([128, 1], mybir.dt.int32)
        nc.gpsimd.iota(iota_i[:], pattern=[[0, 1]], channel_multiplier=1)
        iota_f = const.tile([128, 1], f32)
        nc.vector.tensor_copy(iota_f[:], iota_i[:])
        ones_col = const.tile([128, 1], f32)
        nc.vector.memset(ones_col[:], 1.0)
        # one-hot masks per 512-literal tile (DVE, pre-gather window)
        oh_a, oh_b, oh_l = [], [], []
        for t in range(PE_TILES):
            sl = slice(512 * t, 512 * (t + 1))
            oa = const.tile([128, 512], f32, tag=f"oha{t}")
            nc.vector.tensor_scalar(oa[:], hi_bc[:, sl], iota_f[:, 0:1],
                                    None, OP.is_equal)
            ob = const.tile([128, 512], f32, tag=f"ohb{t}")
            nc.vector.tensor_scalar(ob[:], hib_bc[:, sl], iota_f[:, 0:1],
                                    None, OP.is_equal)
            ol = const.tile([128, 512], f32, tag=f"ohl{t}")
            nc.vector.tensor_scalar(ol[:], lo_bc[:, sl], iota_f[:, 0:1],
                                    None, OP.is_equal)
            oh_a.append(oa); oh_b.append(ob); oh_l.append(ol)
        # X2[k, m] = tab[128k + m] laid out across partitions
        x2a = const.tile([128, RADIX], f32)
        x2b = const.tile([29, RADIX], f32)
        tapr = tab[:].ap[0][0]
        nc.sync.dma_start(
            out=x2a[:],
            in_=bass.AP(tensor=tab[:].tensor, offset=tab[:].offset,
                        ap=[[tapr, 1], [1, 128 * RADIX]]))
        nc.sync.dma_start(
            out=x2b[:],
            in_=bass.AP(tensor=tab[:].tensor,
                        offset=tab[:].offset + 128 * RADIX,
                        ap=[[tapr, 1], [1, 29 * RADIX]]))
        # stage 1+2: Y = X2.T @ onehot_hi ; z = sum_p(Y * onehot_lo)
        zrow = const.tile([1, PE_LP], f32)
        for t in range(PE_TILES):
            Y = pepsum.tile([128, 512], f32, tag="Y")
            nc.tensor.matmul(Y[:], x2a[:], oh_a[t][:],
                             start=True, stop=False)
            nc.tensor.matmul(Y[:], x2b[:], oh_b[t][0:29, :],
                             start=False, stop=True)
            m_sb = work.tile([128, 512], f32, tag="msb")
            nc.vector.tensor_tensor(m_sb[:], Y[:], oh_l[t][:], OP.mult)
            zr = pepsum.tile([1, 512], f32, tag="zr")
            nc.tensor.matmul(zr[0:1, :], ones_col[:], m_sb[:],
                             start=True, stop=True)
            nc.scalar.activation(zrow[0:1, 512 * t:512 * (t + 1)],
                                 zr[0:1, :], AF.Copy)
        # products + (1 - .) on the single-partition row
        perow = const.tile([1, PE_C], f32)
        nc.vector.tensor_tensor(perow[0:1, :], zrow[0:1, 0:PE_L:3],
                                zrow[0:1, 1:PE_L:3], OP.mult)
        nc.vector.scalar_tensor_tensor(perow[0:1, :], perow[0:1, :], 1.0,
                                       zrow[0:1, 2:PE_L:3],
                                       OP.mult, OP.mult)
        nc.vector.tensor_scalar(perow[0:1, :], perow[0:1, :], -1.0, 1.0,
                                OP.mult, OP.add)
        # roundtrip through DRAM to broadcast across partitions
        drow = dpool.tile([1, PE_C], f32)
        nc.scalar.dma_start(out=drow[0:1, :], in_=perow[0:1, :])

        for h in range(H):
            CPG, LPC, LPC_PAD = CPGS[h], LPCS[h], LPC_PADS[h]
            C_CHUNK, C_OFF = C_CHUNKS[h], C_OFFS[h]
            z = work.tile([128, max(LPC_PADS)], f32, tag="z")
            nc.gpsimd.ap_gather(
                z[:, 0:LPC_PAD], tab[:],
                idx_sb[:, COL_OFFS[h]:COL_OFFS[h] + COLS_HS[h]],
                channels=128, num_elems=2 * NV, d=1, num_idxs=LPC_PAD)

            p01 = work.tile([128, max(CPGS)], f32, tag="p01")
            nc.vector.tensor_tensor(p01[:, 0:CPG], z[:, 0:LPC:3],
                                    z[:, 1:LPC:3], OP.mult)
            r = work.tile([128, max(CPGS)], f32, tag="r")
            # r = z0 z1 z2 (the 1 - . fold happens in the ACT copy)
            nc.vector.scalar_tensor_tensor(r[:, 0:CPG], p01[:, 0:CPG],
                                           1.0, z[:, 2:LPC:3],
                                           OP.mult, OP.mult)

            # PE broadcast: group g's (16-replicated) row -> all 128
            # partitions.  sum over the 16 identical values * 1/16 is
            # bitwise exact.
            P = psum.tile([128, GROUPS, PBLK], f32, tag="P")
            for g in range(GROUPS):
                nc.tensor.matmul(P[:, g, 0:CPG], sel[:, g, :],
                                 r[:, 0:CPG], start=True, stop=True)
            # pack the 8 group blocks contiguously so output descriptors
            # are C_CHUNK*4 bytes
            bcast = work.tile([128, GROUPS * max(CPGS)], f32, tag="bcast")
            bt = bcast[:]
            prow = bt.ap[0][0]
            bview = bass.AP(tensor=bt.tensor, offset=bt.offset,
                            ap=[[prow, 128], [CPG, GROUPS], [1, CPG]])
            # bcast = Copy(-P + 1) = 1 - z0 z1 z2
            nc.scalar.activation(bview, P[:, :, 0:CPG], AF.Copy,
                                 scale=-1.0, bias=1.0)

            out_w = C_CHUNK
            if h == H - 1:
                # append the PE-gathered tail columns via a stride-0
                # broadcast read of the DRAM row
                peb = bass.AP(tensor=bt.tensor, offset=bt.offset + C_CHUNK,
                              ap=[[prow, 128], [1, PE_C]])
                dr = drow[0:1, :]
                nc.scalar.dma_start(
                    out=peb,
                    in_=bass.AP(tensor=dr.tensor, offset=dr.offset,
                                ap=[[0, 128], [1, PE_C]]))
                out_w = C_CHUNK + PE_C

            # 8 row-block output DMAs, 128 rows each, spread across both
            # HWDGE rings
            bap = bass.AP(tensor=bt.tensor, offset=bt.offset,
                          ap=[[prow, 128], [1, out_w]])
            for blk in range(8):
                dst = bass.AP(tensor=out_d,
                              offset=blk * 128 * C_PAD + C_OFF,
                              ap=[[C_PAD, 128], [1, out_w]])
                rings[blk % 2].dma_start(out=dst, in_=bap)
    nc.compile()
    return nc


def _prep_indices(clause_idx, clause_sign):
    """Per-core wrapped int16 combined-index arrays [128, IDX_COLS].

    Literal order per group g: chunk-major — for chunk h, group g owns
    core clauses [C_CHUNK*h + CPG*g, C_CHUNK*h + CPG*(g+1)), padded to
    LPC_PAD literals per (group, chunk) block.
    """
    idx2 = clause_idx.astype(np.int32) + NV * (clause_sign <= 0.0)
    idx2 = idx2.astype(np.int16)
    per_core = []
    for c in range(NCORES):
        cl = idx2[c * C_CORE:(c + 1) * C_CORE]            # [5250, 3]
        buf = np.zeros((C_PAD, KLIT), dtype=np.int16)
        buf[:cl.shape[0]] = cl
        # group g's stream = concat over chunks of its padded block
        gs = np.zeros((GROUPS, IDX_COLS * 16), dtype=np.int16)
        for h in range(H):
            blk = buf[C_OFFS[h]:C_OFFS[h] + C_CHUNKS[h]]  # [8*CPG, 3]
            blk = blk.reshape(GROUPS, LPCS[h])
            o = COL_OFFS[h] * 16
            gs[:, o:o + LPCS[h]] = blk
        # wrap: literal j at partition 16g + j%16, col j//16
        w = (gs.reshape(GROUPS, IDX_COLS, 16)
               .transpose(0, 2, 1)
               .reshape(128, IDX_COLS))
        # PE tail: radix-decomposed literals, plain order, f32 rows
        pe = buf[PE_OFF:PE_OFF + PE_C].reshape(-1).astype(np.int32)
        pe = np.concatenate([pe, np.zeros(PE_LP - PE_L, np.int32)])
        hi = pe // RADIX
        hia = hi.astype(np.float32)[None, :]
        hib = (hi - 128).astype(np.float32)[None, :]
        lo = (pe % RADIX).astype(np.float32)[None, :]
        per_core.append((np.ascontiguousarray(w), hia, hib, lo))
    return per_core


def _ensure_ntff_hook():
    """The agent image lacks antenv.axon_hooks; synthesize it so
    run_bass_kernel_spmd(trace=True) can capture NTFF profiles."""
    import sys, types
    try:
        from antenv import axon_hooks  # noqa: F401
        return
    except ImportError:
        pass
    m = types.ModuleType("antenv.axon_hooks")
    _hook = [None]
    m.set_axon_ntff_profile_hook = lambda h: _hook.__setitem__(0, h)
    m.get_axon_ntff_profile_hook = lambda: _hook[0]
    sys.modules["antenv.axon_hooks"] = m
    import antenv
    antenv.axon_hooks = m
    from trn_agent_boot.trn_boot import _ntff_profile_via_ctypes
    m.set_axon_ntff_profile_hook(
        _ntff_profile_via_ctypes("/opt/axon/libaxon_pjrt.so"))


def _run(emb, idx_cores, trace=False):
    from concourse.bass_utils import run_bass_kernel_spmd
    if trace:
        _ensure_ntff_hook()
    if "prog" not in _CACHE:
        _CACHE["prog"] = _build()
    nc = _CACHE["prog"]
    in_maps = [{"emb": emb, "idxw": idx_cores[c][0],
                "hia": idx_cores[c][1], "hib": idx_cores[c][2],
                "lo": idx_cores[c][3]} for c in range(NCORES)]
    return run_bass_kernel_spmd(nc, in_maps, list(range(NCORES)),
                                trace=trace)


def kernel(input_idx=None, emb_weight=None, clause_idx=None,
           clause_sign=None, _trace=False, _want_results=False):
    emb = np.ascontiguousarray(np.asarray(emb_weight, dtype=np.float32))
    cidx = np.asarray(clause_idx, dtype=np.int32)
    csgn = np.asarray(clause_sign, dtype=np.float32)
    idx_cores = _prep_indices(cidx, csgn)
    res = _run(emb, idx_cores, trace=_trace)
    full = np.empty((B, C_TOTAL), dtype=np.float32)
    for c in range(NCORES):
        full[:, c * C_CORE:(c + 1) * C_CORE] = \
            res.results[c]["out"][:, :C_CORE]
    if _want_results:
        return full, res
    return full



# revision 7
# speedup vs baseline: 1.6140x; 1.6140x over previous
"""Trainium2 Bass kernel: batched soft 3-SAT circuit evaluation.

out[b, c] = 1 - prod_k w[c,k],  w = (sign>0 ? 1-x : x)[idx],
x = sigmoid(emb[0]).  Every batch row is identical (input_idx is all
zeros, the embedding has a single row, and jnp.take clamps OOB), so the
device computes each clause result once and broadcast-writes the rows.

Sharding: clauses split across 8 NeuronCores (5250 each, padded to
5376 = 42*128).  Host work is index-layout prep only (fold the sign
into a combined table offset, pad, lay literals out per-partition) plus
dequantize/concat of per-core outputs.

Per-core device pipeline (v4 — indirect-DMA gather, u8 output):
  prologue: load emb wrapped [16, 625]; ACT sigmoid -> x; DVE 1-x;
    write combined table [1-x | x] (20000 f32) back to DRAM.
  per chunk (42 j-slots split [11, 11, 10, 10]; clause c = 128j + p
  lives at partition p = c%128, slot j = c//128):
    - indirect-DMA gather: w[128, 3*jw] f32 <- tab[offs] (one HW
      descriptor per literal, offsets shipped from host as int32)
    - DVE: r = w0*w1*w2 -> bf16 [128, jw]
    - PE transpose: T[jw, 128] (psum, bf16)
    - ACT: per-j copy T[j,:] into block-diagonal rhs[j, 128j:128j+128]
    - PE: one matmul ones[jw,128]^T @ rhs -> psum P[128, 128*jw]
      (replicates clause row into all 128 partitions)
    - ACT: u8 quantize bcast[:, 128*j0+...] = 255*(1-P) + 0.5
  epilogue: 8 row-block DMAs bcast[128, 5376]u8 -> out[1024, 5376]u8
    (full 5376-byte row descriptors, sync/scalar rings).
Host divides by 255 to produce f32.
"""

import numpy as np

NV = 10000
C_TOTAL = 42000
KLIT = 3
B = 1024
NCORES = 8
C_CORE = C_TOTAL // NCORES     # 5250
JW = 42                        # j-slots per partition; C_PAD = 42*128
C_PAD = JW * 128               # 5376
CHUNKS = [11, 11, 10, 10]      # j-slots per pipeline chunk
NIDX = JW * KLIT               # 126 offsets per partition

_CACHE = {}


def _build():
    import concourse.bass as bass
    import concourse.tile as tile
    from concourse import bacc, mybir
    from concourse.masks import make_identity
    from contextlib import ExitStack

    f32 = mybir.dt.float32
    bf16 = mybir.dt.bfloat16
    u8 = mybir.dt.uint8
    i32 = mybir.dt.int32
    AF = mybir.ActivationFunctionType
    OP = mybir.AluOpType

    nc = bacc.Bacc("TRN2", target_bir_lowering=False, debug=False,
                   num_devices=NCORES)
    emb_d = nc.dram_tensor("emb", [1, NV], f32, kind="ExternalInput")
    off_d = nc.dram_tensor("offs", [128, NIDX], i32, kind="ExternalInput")
    out_d = nc.dram_tensor("out", [B, C_PAD], u8, kind="ExternalOutput")

    with tile.TileContext(nc) as tc, ExitStack() as ctx:
        const = ctx.enter_context(tc.tile_pool(name="const", bufs=1))
        work = ctx.enter_context(tc.tile_pool(name="work", bufs=2))
        psum = ctx.enter_context(
            tc.tile_pool(name="psum", bufs=2, space="PSUM"))
        dpool = ctx.enter_context(
            tc.tile_pool(name="dram", bufs=1, space="DRAM"))

        # ---- prologue: sigmoid table -> DRAM [1-x | x], f32 ----
        tab = dpool.tile([2 * NV, 1], f32)
        assert tab[:].offset == 0, "indirect DMA needs offset-0 table"

        offs_sb = const.tile([128, NIDX], i32)
        nc.sync.dma_start(out=offs_sb[:], in_=off_d[:, :])

        ew = const.tile([16, 625], f32)
        nc.sync.dma_start(
            out=ew[:],
            in_=bass.AP(tensor=emb_d, offset=0, ap=[[625, 16], [1, 625]]))
        xw = const.tile([16, 625], f32)
        nc.scalar.activation(xw[:], ew[:], AF.Sigmoid)
        yw = const.tile([16, 625], f32)
        nc.vector.tensor_scalar(yw[:], xw[:], -1.0, 1.0, OP.mult, OP.add)
        # combined table: rows [0:NV) = 1-x (sign>0), [NV:2NV) = x
        tt = tab[:]
        nc.scalar.dma_start(
            out=bass.AP(tensor=tt.tensor, offset=0,
                        ap=[[625, 16], [1, 625]]),
            in_=yw[:])
        nc.scalar.dma_start(
            out=bass.AP(tensor=tt.tensor, offset=NV,
                        ap=[[625, 16], [1, 625]]),
            in_=xw[:])

        ident = const.tile([128, 128], u8)
        make_identity(nc, ident[:])
        ones128 = const.tile([128, 128], bf16)
        nc.vector.memset(ones128[:], 1.0)
        diag_a = const.tile([128, 128], bf16)
        nc.vector.memset(diag_a[:], 0.0)
        diag_b = const.tile([128, 128], bf16)
        nc.vector.memset(diag_b[:], 0.0)
        diags = [diag_a, diag_b]
        bcast = const.tile([128, C_PAD], u8)

        j0 = 0
        for h, jw in enumerate(CHUNKS):
            lw = 3 * jw
            # stride-2 dest -> one 4B descriptor per literal (the DGE
            # coalesces contiguous runs; each run consumes ONE offset)
            w = work.tile([128, 6 * max(CHUNKS)], f32, tag="w")
            wt = w[:]
            wprow = wt.ap[0][0]
            nc.gpsimd.indirect_dma_start(
                out=bass.AP(tensor=wt.tensor, offset=wt.offset,
                            ap=[[wprow, 128], [2, lw], [1, 1]]),
                out_offset=None,
                in_=tab[:, :],
                in_offset=bass.IndirectOffsetOnAxis(
                    ap=offs_sb[:, 3 * j0:3 * j0 + lw], axis=0))

            p01 = work.tile([128, max(CHUNKS)], f32, tag="p01")
            nc.vector.tensor_tensor(p01[:, 0:jw], w[:, 0:6 * jw:6],
                                    w[:, 2:6 * jw:6], OP.mult)
            r = work.tile([128, max(CHUNKS)], bf16, tag="r")
            nc.vector.tensor_tensor(r[:, 0:jw], p01[:, 0:jw],
                                    w[:, 4:6 * jw:6], OP.mult)

            P2 = psum.tile([128, 128 * max(CHUNKS)], f32, tag="P2")
            for j in range(jw):
                dg = diags[(j0 + j) % 2]
                nc.vector.copy_predicated(
                    dg[:], ident[:],
                    r[:, j:j + 1].to_broadcast([128, 128]))
                nc.tensor.matmul(P2[:, 128 * j:128 * (j + 1)],
                                 ones128[:], dg[:], start=True, stop=True)
            # u8 quantize: out = 255*(1 - r) + 0.5, truncated to u8
            nc.scalar.activation(
                bcast[:, 128 * j0:128 * (j0 + jw)], P2[:, 0:128 * jw],
                AF.Copy, scale=-255.0, bias=255.5)
            j0 += jw

        # ---- output: 8 row blocks, full 5376B row descriptors ----
        bt = bcast[:]
        prow = bt.ap[0][0]
        src = bass.AP(tensor=bt.tensor, offset=bt.offset,
                      ap=[[prow, 128], [1, C_PAD]])
        rings = [nc.sync, nc.scalar]
        for blk in range(8):
            dst = bass.AP(tensor=out_d, offset=blk * 128 * C_PAD,
                          ap=[[C_PAD, 128], [1, C_PAD]])
            rings[blk % 2].dma_start(out=dst, in_=src)
    nc.compile()
    return nc


def _prep_offsets(clause_idx, clause_sign):
    """Per-core [128, NIDX] int32 combined-table offsets.

    Clause c (core-local, padded order) lives at partition c%128,
    j-slot c//128; its literal k is offset column 3*(c//128) + k.
    offset = idx + NV * (sign <= 0):  sign>0 -> rows [0:NV) = 1-x,
    else rows [NV:2NV) = x.
    """
    v = clause_idx.astype(np.int64) + NV * (clause_sign <= 0.0)
    v = v.astype(np.int32)
    per_core = []
    for c in range(NCORES):
        cl = v[c * C_CORE:(c + 1) * C_CORE]            # [5250, 3]
        buf = np.zeros((C_PAD, KLIT), dtype=np.int32)
        buf[:cl.shape[0]] = cl
        offs = (buf.reshape(JW, 128, KLIT)
                   .transpose(1, 0, 2)
                   .reshape(128, NIDX))
        per_core.append(np.ascontiguousarray(offs))
    return per_core


def _ensure_ntff_hook():
    """The agent image lacks antenv.axon_hooks; synthesize it so
    run_bass_kernel_spmd(trace=True) can capture NTFF profiles."""
    import sys, types
    try:
        from antenv import axon_hooks  # noqa: F401
        return
    except ImportError:
        pass
    m = types.ModuleType("antenv.axon_hooks")
    _hook = [None]
    m.set_axon_ntff_profile_hook = lambda h: _hook.__setitem__(0, h)
    m.get_axon_ntff_profile_hook = lambda: _hook[0]
    sys.modules["antenv.axon_hooks"] = m
    import antenv
    antenv.axon_hooks = m
    from trn_agent_boot.trn_boot import _ntff_profile_via_ctypes
    m.set_axon_ntff_profile_hook(
        _ntff_profile_via_ctypes("/opt/axon/libaxon_pjrt.so"))


def _run(emb, offs_cores, trace=False):
    from concourse.bass_utils import run_bass_kernel_spmd
    if trace:
        _ensure_ntff_hook()
    if "prog" not in _CACHE:
        _CACHE["prog"] = _build()
    nc = _CACHE["prog"]
    in_maps = [{"emb": emb, "offs": offs_cores[c]} for c in range(NCORES)]
    return run_bass_kernel_spmd(nc, in_maps, list(range(NCORES)),
                                trace=trace)


def kernel(input_idx=None, emb_weight=None, clause_idx=None,
           clause_sign=None, _trace=False, _want_results=False):
    emb = np.ascontiguousarray(np.asarray(emb_weight, dtype=np.float32))
    cidx = np.asarray(clause_idx, dtype=np.int32)
    csgn = np.asarray(clause_sign, dtype=np.float32)
    offs_cores = _prep_offsets(cidx, csgn)
    res = _run(emb, offs_cores, trace=_trace)
    full = np.empty((B, C_TOTAL), dtype=np.float32)
    inv = np.float32(1.0 / 255.0)
    for c in range(NCORES):
        q = res.results[c]["out"][:, :C_CORE]
        full[:, c * C_CORE:(c + 1) * C_CORE] = q.astype(np.float32) * inv
    if _want_results:
        return full, res
    return full


# revision 8
# speedup vs baseline: 1.9924x; 1.2345x over previous
"""Trainium2 Bass kernel v5: batched soft 3-SAT via PE-radix gather.

out[b, c] = 1 - prod_k w[c,k],  w = sign>0 ? 1-x : x,  x = sigmoid(emb[0]).
All batch rows are identical, so each clause result is computed once and
broadcast-written.

The literal "gather" runs on the tensor engine as two matmuls per tile
(radix-128 decomposition v = 128*hi + lo, hi < 79):
  stage 1:  Y[m, j] = sum_k x2[k, m] * hih[k, j] = x[128*hi_j + m] + gamma_j
            (x2[k, m] = x[128k + m], row 79 = ones; hih = one-hot of hi
             with row 79 carrying gamma = -1 if sign>0 else 0)
  stage 2:  W[m, j] = sum_p ones[p, m] * (Y[p, j] * slo[p, j])
            = sigma_j * (x_j + sigma_j*c_j) = w_j   for ALL m
            (slo = sigma at row lo_j; the collapse IS the 128-row
             broadcast)
then DVE forms r = w0*w1*w2 per clause and ACT quantizes
255*(1-r)+0.5 into a u8 row replicated in all 128 partitions; 16
row-block DMAs (full/half 1024-row blocks) write [1024, 5376] u8.
Host divides by 255 (u8 step 1/255 = 0.2% of absmax vs 2% tolerance).

Sharding: clauses across 8 cores (5250 each, padded 5376 = 32 tiles x
168 clauses). Host prep is index layout only: one-hot rows hih (int8)
and sigma-hot rows slo (fp16) shipped per core.
"""

import numpy as np

NV = 10000
C_TOTAL = 42000
KLIT = 3
B = 1024
NCORES = 8
C_CORE = C_TOTAL // NCORES     # 5250
TILES = 32
CPT = 168                      # clauses per tile
C_PAD = TILES * CPT            # 5376
LPT = CPT * KLIT               # 504 literals per tile
L_ALL = TILES * LPT            # 16128
KHI = 80                       # 79 x-chunks + gamma/ones row

_CACHE = {}


def _build():
    import concourse.bass as bass
    import concourse.tile as tile
    from concourse import bacc, mybir
    from contextlib import ExitStack

    f32 = mybir.dt.float32
    fp16 = mybir.dt.float16
    u8 = mybir.dt.uint8
    i8 = mybir.dt.int8
    AF = mybir.ActivationFunctionType
    OP = mybir.AluOpType

    nc = bacc.Bacc("TRN2", target_bir_lowering=False, debug=False,
                   num_devices=NCORES)
    emb_d = nc.dram_tensor("emb", [1, NV], f32, kind="ExternalInput")
    hih_d = nc.dram_tensor("hih", [KHI, L_ALL], i8, kind="ExternalInput")
    slo_d = nc.dram_tensor("slo", [128, L_ALL], fp16, kind="ExternalInput")
    out_d = nc.dram_tensor("out", [B, C_PAD], u8, kind="ExternalOutput")

    with tile.TileContext(nc) as tc, ExitStack() as ctx:
        const = ctx.enter_context(tc.tile_pool(name="const", bufs=1))
        work = ctx.enter_context(tc.tile_pool(name="work", bufs=2))
        psum = ctx.enter_context(
            tc.tile_pool(name="psum", bufs=2, space="PSUM"))

        # ---- x2 table: x2[k, m] = sigmoid(emb[128k + m]), row 79 = 1 ----
        ew = const.tile([KHI - 1, 128], f32)
        nc.vector.memset(ew[:], 0.0)
        nc.sync.dma_start(
            out=ew[0:78, :],
            in_=bass.AP(tensor=emb_d, offset=0, ap=[[128, 78], [1, 128]]))
        nc.sync.dma_start(
            out=ew[78:79, 0:16],
            in_=bass.AP(tensor=emb_d, offset=9984, ap=[[16, 1], [1, 16]]))
        x2 = const.tile([KHI, 128], fp16)
        nc.vector.memset(x2[:], 1.0)
        nc.scalar.activation(x2[0:79, :], ew[:], AF.Sigmoid)

        ones128 = const.tile([128, 128], fp16)
        nc.vector.memset(ones128[:], 1.0)
        bcast = const.tile([128, C_PAD], u8)

        # ---- one-hot inputs, loaded in halves on the SWDGE ring ----
        hih = const.tile([KHI, L_ALL], i8)
        slo = const.tile([128, L_ALL], fp16)
        HL = L_ALL // 2
        for half in range(2):
            sl = slice(half * HL, (half + 1) * HL)
            nc.gpsimd.dma_start(out=hih[:, sl], in_=hih_d[:, sl])
            nc.gpsimd.dma_start(out=slo[:, sl], in_=slo_d[:, sl])

        for t in range(TILES):
            lsl = slice(LPT * t, LPT * (t + 1))
            ohb = work.tile([KHI, LPT], fp16, tag="ohb")
            cvt = nc.vector if t % 2 == 0 else nc.scalar
            if cvt is nc.vector:
                nc.vector.tensor_copy(ohb[:], hih[:, lsl])
            else:
                nc.scalar.activation(ohb[:], hih[:, lsl], AF.Copy)

            Y = psum.tile([128, LPT], f32, tag="Y")
            nc.tensor.matmul(Y[:], x2[:], ohb[:], start=True, stop=True)

            m_sb = work.tile([128, LPT], fp16, tag="m_sb")
            nc.vector.tensor_tensor(m_sb[:], Y[:], slo[:, lsl], OP.mult)

            W = psum.tile([128, LPT], f32, tag="W")
            nc.tensor.matmul(W[:], ones128[:], m_sb[:],
                             start=True, stop=True)

            w1 = work.tile([128, CPT], f32, tag="w1")
            nc.vector.tensor_copy(w1[:], W[:, 1:LPT:3])
            p01 = work.tile([128, CPT], f32, tag="p01")
            nc.vector.tensor_tensor(p01[:], W[:, 0:LPT:3], w1[:], OP.mult)
            rr = work.tile([128, CPT], f32, tag="rr")
            nc.vector.tensor_tensor(rr[:], p01[:], W[:, 2:LPT:3], OP.mult)
            # u8 quantize: 255*(1 - r) + 0.5
            nc.scalar.activation(bcast[:, CPT * t:CPT * (t + 1)], rr[:],
                                 AF.Copy, scale=-255.0, bias=255.5)

            # fire the first output column-half once tiles 0..15 done
            if t == TILES // 2 - 1:
                _emit_out(nc, bass, bcast, out_d, 0, C_PAD // 2)
        _emit_out(nc, bass, bcast, out_d, C_PAD // 2, C_PAD // 2)
    nc.compile()
    return nc


def _emit_out(nc, bass, bcast, out_d, col0, width):
    bt = bcast[:]
    prow = bt.ap[0][0]
    src = bass.AP(tensor=bt.tensor, offset=bt.offset + col0,
                  ap=[[prow, 128], [1, width]])
    rings = [nc.sync, nc.scalar]
    for blk in range(8):
        dst = bass.AP(tensor=out_d, offset=blk * 128 * C_PAD + col0,
                      ap=[[C_PAD, 128], [1, width]])
        rings[blk % 2].dma_start(out=dst, in_=src)


def _prep_onehots(clause_idx, clause_sign):
    """Per-core (hih int8 [80, L_ALL], slo fp16 [128, L_ALL])."""
    per_core = []
    for c in range(NCORES):
        v = clause_idx[c * C_CORE:(c + 1) * C_CORE].astype(np.int64)
        s = clause_sign[c * C_CORE:(c + 1) * C_CORE] > 0.0
        vp = np.zeros((C_PAD, KLIT), np.int64)
        sp = np.zeros((C_PAD, KLIT), bool)
        vp[:v.shape[0]] = v
        sp[:s.shape[0]] = s
        vf = vp.reshape(-1)
        sf = sp.reshape(-1)
        hi = (vf >> 7).astype(np.int64)
        lo = (vf & 127).astype(np.int64)
        sigma = np.where(sf, -1.0, 1.0).astype(np.float16)
        gamma = np.where(sf, -1, 0).astype(np.int8)
        cols = np.arange(L_ALL)
        hih = np.zeros((KHI, L_ALL), np.int8)
        hih[hi, cols] = 1
        hih[KHI - 1, :] = gamma
        slo = np.zeros((128, L_ALL), np.float16)
        slo[lo, cols] = sigma
        per_core.append((hih, slo))
    return per_core


def _ensure_ntff_hook():
    """The agent image lacks antenv.axon_hooks; synthesize it so
    run_bass_kernel_spmd(trace=True) can capture NTFF profiles."""
    import sys, types
    try:
        from antenv import axon_hooks  # noqa: F401
        return
    except ImportError:
        pass
    m = types.ModuleType("antenv.axon_hooks")
    _hook = [None]
    m.set_axon_ntff_profile_hook = lambda h: _hook.__setitem__(0, h)
    m.get_axon_ntff_profile_hook = lambda: _hook[0]
    sys.modules["antenv.axon_hooks"] = m
    import antenv
    antenv.axon_hooks = m
    from trn_agent_boot.trn_boot import _ntff_profile_via_ctypes
    m.set_axon_ntff_profile_hook(
        _ntff_profile_via_ctypes("/opt/axon/libaxon_pjrt.so"))


def _run(emb, oh_cores, trace=False):
    from concourse.bass_utils import run_bass_kernel_spmd
    if trace:
        _ensure_ntff_hook()
    if "prog" not in _CACHE:
        _CACHE["prog"] = _build()
    nc = _CACHE["prog"]
    in_maps = [{"emb": emb, "hih": oh_cores[c][0], "slo": oh_cores[c][1]}
               for c in range(NCORES)]
    return run_bass_kernel_spmd(nc, in_maps, list(range(NCORES)),
                                trace=trace)


def kernel(input_idx=None, emb_weight=None, clause_idx=None,
           clause_sign=None, _trace=False, _want_results=False):
    emb = np.ascontiguousarray(np.asarray(emb_weight, dtype=np.float32))
    cidx = np.asarray(clause_idx, dtype=np.int32)
    csgn = np.asarray(clause_sign, dtype=np.float32)
    oh_cores = _prep_onehots(cidx, csgn)
    res = _run(emb, oh_cores, trace=_trace)
    full = np.empty((B, C_TOTAL), dtype=np.float32)
    inv = np.float32(1.0 / 255.0)
    for c in range(NCORES):
        q = res.results[c]["out"][:, :C_CORE]
        full[:, c * C_CORE:(c + 1) * C_CORE] = q.astype(np.float32) * inv
    if _want_results:
        return full, res
    return full


# revision 9
# speedup vs baseline: 2.0266x; 1.0172x over previous
"""Trainium2 Bass kernel v5: batched soft 3-SAT via PE-radix gather.

out[b, c] = 1 - prod_k w[c,k],  w = sign>0 ? 1-x : x,  x = sigmoid(emb[0]).
All batch rows are identical, so each clause result is computed once and
broadcast-written.

The literal "gather" runs on the tensor engine as two matmuls per tile
(radix-128 decomposition v = 128*hi + lo, hi < 79):
  stage 1:  Y[m, j] = sum_k x2[k, m] * hih[k, j] = x[128*hi_j + m] + gamma_j
            (x2[k, m] = x[128k + m], row 79 = ones; hih = one-hot of hi
             with row 79 carrying gamma = -1 if sign>0 else 0)
  stage 2:  W[m, j] = sum_p ones[p, m] * (Y[p, j] * slo[p, j])
            = sigma_j * (x_j + sigma_j*c_j) = w_j   for ALL m
            (slo = sigma at row lo_j; the collapse IS the 128-row
             broadcast)
then DVE forms r = w0*w1*w2 per clause and ACT quantizes
255*(1-r)+0.5 into a u8 row replicated in all 128 partitions; 16
row-block DMAs (full/half 1024-row blocks) write [1024, 5376] u8.
Host divides by 255 (u8 step 1/255 = 0.2% of absmax vs 2% tolerance).

Sharding: clauses across 8 cores (5250 each, padded 5376 = 32 tiles x
168 clauses). Host prep is index layout only: one-hot rows hih (int8)
and sigma-hot rows slo (fp16) shipped per core.
"""

import numpy as np

NV = 10000
C_TOTAL = 42000
KLIT = 3
B = 1024
NCORES = 8
C_CORE = C_TOTAL // NCORES     # 5250
TILES = 32
CPT = 168                      # clauses per tile
C_PAD = TILES * CPT            # 5376
LPT = CPT * KLIT               # 504 literals per tile
L_ALL = TILES * LPT            # 16128
KHI = 80                       # 79 x-chunks + gamma/ones row

_CACHE = {}


def _build():
    import concourse.bass as bass
    import concourse.tile as tile
    from concourse import bacc, mybir
    from contextlib import ExitStack

    f32 = mybir.dt.float32
    fp16 = mybir.dt.float16
    u8 = mybir.dt.uint8
    i8 = mybir.dt.int8
    AF = mybir.ActivationFunctionType
    OP = mybir.AluOpType

    nc = bacc.Bacc("TRN2", target_bir_lowering=False, debug=False,
                   num_devices=NCORES)
    emb_d = nc.dram_tensor("emb", [1, NV], f32, kind="ExternalInput")
    hih_d = nc.dram_tensor("hih", [KHI, L_ALL], fp16, kind="ExternalInput")
    slo_d = nc.dram_tensor("slo", [128, L_ALL], fp16, kind="ExternalInput")
    out_d = nc.dram_tensor("out", [B, C_PAD], u8, kind="ExternalOutput")

    with tile.TileContext(nc) as tc, ExitStack() as ctx:
        const = ctx.enter_context(tc.tile_pool(name="const", bufs=1))
        work = ctx.enter_context(tc.tile_pool(name="work", bufs=2))
        psum = ctx.enter_context(
            tc.tile_pool(name="psum", bufs=2, space="PSUM"))

        # ---- x2 table: x2[k, m] = sigmoid(emb[128k + m]), row 79 = 1 ----
        ew = const.tile([KHI - 1, 128], f32)
        nc.vector.memset(ew[:], 0.0)
        nc.sync.dma_start(
            out=ew[0:78, :],
            in_=bass.AP(tensor=emb_d, offset=0, ap=[[128, 78], [1, 128]]))
        nc.sync.dma_start(
            out=ew[78:79, 0:16],
            in_=bass.AP(tensor=emb_d, offset=9984, ap=[[16, 1], [1, 16]]))
        x2 = const.tile([KHI, 128], fp16)
        nc.vector.memset(x2[:], 1.0)
        nc.scalar.activation(x2[0:79, :], ew[:], AF.Sigmoid)

        ones128 = const.tile([128, 128], fp16)
        nc.vector.memset(ones128[:], 1.0)
        bcast = const.tile([128, C_PAD], u8)

        # ---- one-hot inputs, loaded in halves on the SWDGE ring ----
        hih = const.tile([KHI, L_ALL], fp16)
        slo = const.tile([128, L_ALL], fp16)
        HL = L_ALL // 4
        for q in range(4):
            sl = slice(q * HL, (q + 1) * HL)
            nc.sync.dma_start(out=hih[:, sl], in_=hih_d[:, sl])
            nc.sync.dma_start(out=slo[:, sl], in_=slo_d[:, sl])

        def stage2(t, W):
            # one psum->SBUF copy (fp16), then 16-bit-rate products
            wsb = work.tile([128, LPT], fp16, tag="wsb", name=f"wsb{t}")
            nc.scalar.activation(wsb[:], W[:], AF.Copy)
            p01 = work.tile([128, CPT], fp16, tag="p01", name=f"p01{t}")
            nc.vector.tensor_tensor(p01[:], wsb[:, 0:CPT],
                                    wsb[:, CPT:2 * CPT], OP.mult)
            rr = work.tile([128, CPT], f32, tag="rr", name=f"rr{t}")
            nc.vector.tensor_tensor(rr[:], p01[:],
                                    wsb[:, 2 * CPT:3 * CPT], OP.mult)
            # u8 quantize: 255*(1 - r) + 0.5
            if t % 4 == 3:
                nc.vector.tensor_scalar(bcast[:, CPT * t:CPT * (t + 1)],
                                        rr[:], -255.0, 255.5,
                                        OP.mult, OP.add)
            else:
                nc.scalar.activation(bcast[:, CPT * t:CPT * (t + 1)],
                                     rr[:], AF.Copy,
                                     scale=-255.0, bias=255.5)

        for pt in range(TILES // 2):
            t0, t1 = 2 * pt, 2 * pt + 1
            sl0 = slice(LPT * t0, LPT * (t0 + 1))
            sl1 = slice(LPT * t1, LPT * (t1 + 1))
            # same stationary (x2) back to back, then same (ones)
            Y0 = psum.tile([128, LPT], f32, tag="Y", name=f"Y{t0}")
            nc.tensor.matmul(Y0[:], x2[:], hih[:, sl0],
                             start=True, stop=True)
            Y1 = psum.tile([128, LPT], f32, tag="Y", name=f"Y{t1}")
            nc.tensor.matmul(Y1[:], x2[:], hih[:, sl1],
                             start=True, stop=True)
            m0 = work.tile([128, LPT], fp16, tag="m_sb", name=f"m{t0}")
            nc.vector.tensor_tensor(m0[:], Y0[:], slo[:, sl0], OP.mult)
            m1 = work.tile([128, LPT], fp16, tag="m_sb", name=f"m{t1}")
            nc.vector.tensor_tensor(m1[:], Y1[:], slo[:, sl1], OP.mult)
            W0 = psum.tile([128, LPT], f32, tag="W", name=f"W{t0}")
            nc.tensor.matmul(W0[:], ones128[:], m0[:],
                             start=True, stop=True)
            W1 = psum.tile([128, LPT], f32, tag="W", name=f"W{t1}")
            nc.tensor.matmul(W1[:], ones128[:], m1[:],
                             start=True, stop=True)
            stage2(t0, W0)
            stage2(t1, W1)

            # fire the first output column-half once tiles 0..15 done
            if t1 == TILES // 2 - 1:
                _emit_out(nc, bass, bcast, out_d, 0, C_PAD // 2)
        _emit_out(nc, bass, bcast, out_d, C_PAD // 2, C_PAD // 2)
    nc.compile()
    return nc


def _emit_out(nc, bass, bcast, out_d, col0, width):
    bt = bcast[:]
    prow = bt.ap[0][0]
    src = bass.AP(tensor=bt.tensor, offset=bt.offset + col0,
                  ap=[[prow, 128], [1, width]])
    rings = [nc.sync, nc.gpsimd, nc.sync, nc.scalar,
             nc.sync, nc.gpsimd, nc.sync, nc.scalar]
    for blk in range(8):
        dst = bass.AP(tensor=out_d, offset=blk * 128 * C_PAD + col0,
                      ap=[[C_PAD, 128], [1, width]])
        rings[blk % 8].dma_start(out=dst, in_=src)


def _prep_onehots(clause_idx, clause_sign):
    """Per-core (hih int8 [80, L_ALL], slo fp16 [128, L_ALL])."""
    per_core = []
    for c in range(NCORES):
        v = clause_idx[c * C_CORE:(c + 1) * C_CORE].astype(np.int64)
        s = clause_sign[c * C_CORE:(c + 1) * C_CORE] > 0.0
        vp = np.zeros((C_PAD, KLIT), np.int64)
        sp = np.zeros((C_PAD, KLIT), bool)
        vp[:v.shape[0]] = v
        sp[:s.shape[0]] = s
        # k-major within each 168-clause tile: J = 504t + 168k + c
        vt = vp.reshape(TILES, CPT, KLIT).transpose(0, 2, 1).reshape(-1)
        st_ = sp.reshape(TILES, CPT, KLIT).transpose(0, 2, 1).reshape(-1)
        vf = vt
        sf = st_
        hi = (vf >> 7).astype(np.int64)
        lo = (vf & 127).astype(np.int64)
        sigma = np.where(sf, -1.0, 1.0).astype(np.float16)
        gamma = np.where(sf, -1, 0).astype(np.float16)
        cols = np.arange(L_ALL)
        hih = np.zeros((KHI, L_ALL), np.float16)
        hih[hi, cols] = 1
        hih[KHI - 1, :] = gamma
        slo = np.zeros((128, L_ALL), np.float16)
        slo[lo, cols] = sigma
        per_core.append((hih, slo))
    return per_core


def _ensure_ntff_hook():
    """The agent image lacks antenv.axon_hooks; synthesize it so
    run_bass_kernel_spmd(trace=True) can capture NTFF profiles."""
    import sys, types
    try:
        from antenv import axon_hooks  # noqa: F401
        return
    except ImportError:
        pass
    m = types.ModuleType("antenv.axon_hooks")
    _hook = [None]
    m.set_axon_ntff_profile_hook = lambda h: _hook.__setitem__(0, h)
    m.get_axon_ntff_profile_hook = lambda: _hook[0]
    sys.modules["antenv.axon_hooks"] = m
    import antenv
    antenv.axon_hooks = m
    from trn_agent_boot.trn_boot import _ntff_profile_via_ctypes
    m.set_axon_ntff_profile_hook(
        _ntff_profile_via_ctypes("/opt/axon/libaxon_pjrt.so"))


def _run(emb, oh_cores, trace=False):
    from concourse.bass_utils import run_bass_kernel_spmd
    if trace:
        _ensure_ntff_hook()
    if "prog" not in _CACHE:
        _CACHE["prog"] = _build()
    nc = _CACHE["prog"]
    in_maps = [{"emb": emb, "hih": oh_cores[c][0], "slo": oh_cores[c][1]}
               for c in range(NCORES)]
    return run_bass_kernel_spmd(nc, in_maps, list(range(NCORES)),
                                trace=trace)


def kernel(input_idx=None, emb_weight=None, clause_idx=None,
           clause_sign=None, _trace=False, _want_results=False):
    emb = np.ascontiguousarray(np.asarray(emb_weight, dtype=np.float32))
    cidx = np.asarray(clause_idx, dtype=np.int32)
    csgn = np.asarray(clause_sign, dtype=np.float32)
    oh_cores = _prep_onehots(cidx, csgn)
    res = _run(emb, oh_cores, trace=_trace)
    full = np.empty((B, C_TOTAL), dtype=np.float32)
    inv = np.float32(1.0 / 255.0)
    for c in range(NCORES):
        q = res.results[c]["out"][:, :C_CORE]
        full[:, c * C_CORE:(c + 1) * C_CORE] = q.astype(np.float32) * inv
    if _want_results:
        return full, res
    return full


# revision 11
# speedup vs baseline: 2.0408x; 1.0070x over previous
"""Trainium2 Bass kernel v5: batched soft 3-SAT via PE-radix gather.

out[b, c] = 1 - prod_k w[c,k],  w = sign>0 ? 1-x : x,  x = sigmoid(emb[0]).
All batch rows are identical, so each clause result is computed once and
broadcast-written.

The literal "gather" runs on the tensor engine as two matmuls per tile
(radix-128 decomposition v = 128*hi + lo, hi < 79):
  stage 1:  Y[m, j] = sum_k x2[k, m] * hih[k, j] = x[128*hi_j + m] + gamma_j
            (x2[k, m] = x[128k + m], row 79 = ones; hih = one-hot of hi
             with row 79 carrying gamma = -1 if sign>0 else 0)
  stage 2:  W[m, j] = sum_p ones[p, m] * (Y[p, j] * slo[p, j])
            = sigma_j * (x_j + sigma_j*c_j) = w_j   for ALL m
            (slo = sigma at row lo_j; the collapse IS the 128-row
             broadcast)
then DVE forms r = w0*w1*w2 per clause and ACT quantizes
255*(1-r)+0.5 into a u8 row replicated in all 128 partitions; 16
row-block DMAs (full/half 1024-row blocks) write [1024, 5376] u8.
Host divides by 255 (u8 step 1/255 = 0.2% of absmax vs 2% tolerance).

Sharding: clauses across 8 cores (5250 each, padded 5376 = 32 tiles x
168 clauses). Host prep is index layout only: one-hot rows hih (int8)
and sigma-hot rows slo (fp16) shipped per core.
"""

import numpy as np

NV = 10000
C_TOTAL = 42000
KLIT = 3
B = 1024
NCORES = 8
C_CORE = C_TOTAL // NCORES     # 5250
TILES = 32
CPT = 168                      # clauses per tile
C_PAD = TILES * CPT            # 5376
LPT = CPT * KLIT               # 504 literals per tile
L_ALL = TILES * LPT            # 16128
KHI = 80                       # 79 x-chunks + gamma/ones row

_CACHE = {}


def _build():
    import concourse.bass as bass
    import concourse.tile as tile
    from concourse import bacc, mybir
    from contextlib import ExitStack

    f32 = mybir.dt.float32
    fp16 = mybir.dt.float16
    u8 = mybir.dt.uint8
    i8 = mybir.dt.int8
    AF = mybir.ActivationFunctionType
    OP = mybir.AluOpType

    nc = bacc.Bacc("TRN2", target_bir_lowering=False, debug=False,
                   num_devices=NCORES)
    emb_d = nc.dram_tensor("emb", [1, NV], f32, kind="ExternalInput")
    hih_d = nc.dram_tensor("hih", [KHI, L_ALL], fp16, kind="ExternalInput")
    slo_d = nc.dram_tensor("slo", [128, L_ALL], fp16, kind="ExternalInput")
    out_d = nc.dram_tensor("out", [B, C_PAD], u8, kind="ExternalOutput")

    with tile.TileContext(nc) as tc, ExitStack() as ctx:
        const = ctx.enter_context(tc.tile_pool(name="const", bufs=1))
        work = ctx.enter_context(tc.tile_pool(name="work", bufs=2))
        psum = ctx.enter_context(
            tc.tile_pool(name="psum", bufs=2, space="PSUM"))

        # ---- x2 table: x2[k, m] = sigmoid(emb[128k + m]), row 79 = 1 ----
        ew = const.tile([KHI - 1, 128], f32)
        nc.vector.memset(ew[:], 0.0)
        nc.sync.dma_start(
            out=ew[0:78, :],
            in_=bass.AP(tensor=emb_d, offset=0, ap=[[128, 78], [1, 128]]))
        nc.sync.dma_start(
            out=ew[78:79, 0:16],
            in_=bass.AP(tensor=emb_d, offset=9984, ap=[[16, 1], [1, 16]]))
        x2 = const.tile([KHI, 128], fp16)
        nc.vector.memset(x2[:], 1.0)
        nc.scalar.activation(x2[0:79, :], ew[:], AF.Sigmoid)

        ones128 = const.tile([128, 128], fp16)
        nc.vector.memset(ones128[:], 1.0)
        bcast = const.tile([128, C_PAD], u8)

        # ---- one-hot inputs, loaded in halves on the SWDGE ring ----
        hih = const.tile([KHI, L_ALL], fp16)
        slo = const.tile([128, L_ALL], fp16)
        HL = L_ALL // 4
        for q in range(4):
            sl = slice(q * HL, (q + 1) * HL)
            nc.sync.dma_start(out=hih[:, sl], in_=hih_d[:, sl])
            nc.sync.dma_start(out=slo[:, sl], in_=slo_d[:, sl])

        def stage2(t, W, cw=CPT):
            # one psum->SBUF copy (fp16), then 16-bit-rate products
            wsb = work.tile([128, LPT], fp16, tag="wsb", name=f"wsb{t}")
            nc.scalar.activation(wsb[:, 0:3 * cw], W[:, 0:3 * cw], AF.Copy)
            p01 = work.tile([128, CPT], fp16, tag="p01", name=f"p01{t}")
            nc.vector.tensor_tensor(p01[:, 0:cw], wsb[:, 0:cw],
                                    wsb[:, cw:2 * cw], OP.mult)
            rr = work.tile([128, CPT], f32, tag="rr", name=f"rr{t}")
            nc.vector.tensor_tensor(rr[:, 0:cw], p01[:, 0:cw],
                                    wsb[:, 2 * cw:3 * cw], OP.mult)
            # u8 quantize: 255*(1 - r) + 0.5
            if t % 4 == 3:
                nc.vector.tensor_scalar(bcast[:, CPT * t:CPT * t + cw],
                                        rr[:, 0:cw], -255.0, 255.5,
                                        OP.mult, OP.add)
            else:
                nc.scalar.activation(bcast[:, CPT * t:CPT * t + cw],
                                     rr[:, 0:cw], AF.Copy,
                                     scale=-255.0, bias=255.5)

        for qt in range(TILES // 4):
            ts = [4 * qt + i for i in range(4)]
            sls = [slice(LPT * t, LPT * (t + 1)) for t in ts]
            lw = {31: 126}
            Ys = []
            for t, sl in zip(ts, sls):
                L = lw.get(t, LPT)
                sl = slice(sl.start, sl.start + L)
                Y = psum.tile([128, LPT], f32, tag="Y", bufs=4,
                              name=f"Y{t}")
                nc.tensor.matmul(Y[:, 0:L], x2[:], hih[:, sl],
                                 start=True, stop=True)
                Ys.append(Y)
            ms = []
            for t, sl, Y in zip(ts, sls, Ys):
                L = lw.get(t, LPT)
                sl = slice(sl.start, sl.start + L)
                m = work.tile([128, LPT], fp16, tag="m_sb", bufs=4,
                              name=f"m{t}")
                nc.vector.tensor_tensor(m[:, 0:L], Y[:, 0:L],
                                        slo[:, sl], OP.mult)
                ms.append(m)
            Ws = []
            for t, m in zip(ts, ms):
                L = lw.get(t, LPT)
                W = psum.tile([128, LPT], f32, tag="W", bufs=4,
                              name=f"W{t}")
                nc.tensor.matmul(W[:, 0:L], ones128[:], m[:, 0:L],
                                 start=True, stop=True)
                Ws.append(W)
            for t, W in zip(ts, Ws):
                stage2(t, W, lw.get(t, LPT) // 3)

            # staged output: half at t15, then 2016 cols at t27
            if ts[-1] == 15:
                _emit_out(nc, bass, bcast, out_d, 0, 2688, 0)
            elif ts[-1] == 27:
                _emit_out(nc, bass, bcast, out_d, 2688, 2016, 0)
        _emit_out(nc, bass, bcast, out_d, 4704, 546, 1)
    nc.compile()
    return nc


def _emit_out(nc, bass, bcast, out_d, col0, width, piece):
    bt = bcast[:]
    prow = bt.ap[0][0]
    src = bass.AP(tensor=bt.tensor, offset=bt.offset + col0,
                  ap=[[prow, 128], [1, width]])
    ring_sets = [
        [nc.sync, nc.gpsimd, nc.sync, nc.scalar,
         nc.sync, nc.gpsimd, nc.sync, nc.scalar],
        [nc.gpsimd, nc.sync, nc.gpsimd, nc.scalar,
         nc.gpsimd, nc.sync, nc.gpsimd, nc.scalar],
    ]
    rings = ring_sets[piece]
    for blk in range(8):
        dst = bass.AP(tensor=out_d, offset=blk * 128 * C_PAD + col0,
                      ap=[[C_PAD, 128], [1, width]])
        rings[blk % 8].dma_start(out=dst, in_=src)


def _prep_onehots(clause_idx, clause_sign):
    """Per-core (hih int8 [80, L_ALL], slo fp16 [128, L_ALL])."""
    per_core = []
    for c in range(NCORES):
        v = clause_idx[c * C_CORE:(c + 1) * C_CORE].astype(np.int64)
        s = clause_sign[c * C_CORE:(c + 1) * C_CORE] > 0.0
        vp = np.zeros((C_PAD, KLIT), np.int64)
        sp = np.zeros((C_PAD, KLIT), bool)
        vp[:v.shape[0]] = v
        sp[:s.shape[0]] = s
        # k-major within each 168-clause tile: J = 504t + 168k + c;
        # tile 31 is trimmed to 42 real clauses laid out 42 apart
        vt = vp.reshape(TILES, CPT, KLIT).transpose(0, 2, 1).reshape(-1)
        st_ = sp.reshape(TILES, CPT, KLIT).transpose(0, 2, 1).reshape(-1)
        base = 504 * 31
        vt[base:] = 0
        st_[base:] = False
        for k in range(KLIT):
            vt[base + 42 * k:base + 42 * (k + 1)] = vp[5208:5250, k]
            st_[base + 42 * k:base + 42 * (k + 1)] = sp[5208:5250, k]
        vf = vt
        sf = st_
        hi = (vf >> 7).astype(np.int64)
        lo = (vf & 127).astype(np.int64)
        sigma = np.where(sf, -1.0, 1.0).astype(np.float16)
        gamma = np.where(sf, -1, 0).astype(np.float16)
        cols = np.arange(L_ALL)
        hih = np.zeros((KHI, L_ALL), np.float16)
        hih[hi, cols] = 1
        hih[KHI - 1, :] = gamma
        slo = np.zeros((128, L_ALL), np.float16)
        slo[lo, cols] = sigma
        per_core.append((hih, slo))
    return per_core


def _ensure_ntff_hook():
    """The agent image lacks antenv.axon_hooks; synthesize it so
    run_bass_kernel_spmd(trace=True) can capture NTFF profiles."""
    import sys, types
    try:
        from antenv import axon_hooks  # noqa: F401
        return
    except ImportError:
        pass
    m = types.ModuleType("antenv.axon_hooks")
    _hook = [None]
    m.set_axon_ntff_profile_hook = lambda h: _hook.__setitem__(0, h)
    m.get_axon_ntff_profile_hook = lambda: _hook[0]
    sys.modules["antenv.axon_hooks"] = m
    import antenv
    antenv.axon_hooks = m
    from trn_agent_boot.trn_boot import _ntff_profile_via_ctypes
    m.set_axon_ntff_profile_hook(
        _ntff_profile_via_ctypes("/opt/axon/libaxon_pjrt.so"))


def _run(emb, oh_cores, trace=False):
    from concourse.bass_utils import run_bass_kernel_spmd
    if trace:
        _ensure_ntff_hook()
    if "prog" not in _CACHE:
        _CACHE["prog"] = _build()
    nc = _CACHE["prog"]
    in_maps = [{"emb": emb, "hih": oh_cores[c][0], "slo": oh_cores[c][1]}
               for c in range(NCORES)]
    return run_bass_kernel_spmd(nc, in_maps, list(range(NCORES)),
                                trace=trace)


def kernel(input_idx=None, emb_weight=None, clause_idx=None,
           clause_sign=None, _trace=False, _want_results=False):
    emb = np.ascontiguousarray(np.asarray(emb_weight, dtype=np.float32))
    cidx = np.asarray(clause_idx, dtype=np.int32)
    csgn = np.asarray(clause_sign, dtype=np.float32)
    oh_cores = _prep_onehots(cidx, csgn)
    res = _run(emb, oh_cores, trace=_trace)
    full = np.empty((B, C_TOTAL), dtype=np.float32)
    inv = np.float32(1.0 / 255.0)
    for c in range(NCORES):
        q = res.results[c]["out"][:, :C_CORE]
        full[:, c * C_CORE:(c + 1) * C_CORE] = q.astype(np.float32) * inv
    if _want_results:
        return full, res
    return full
